# revision 1
# baseline (speedup 1.0000x reference)
"""Multi-head attention (QK-LayerNorm, causal) Trainium2 kernel over 8 NeuronCores.

Sharding: tensor-parallel over heads — 2 heads per core. Each core computes
q/k/v projections for its 128 channels, per-head attention for both batches,
and a partial output projection (its 128-channel slice of Wo); the host sums
the 8 partial projections.

Device-side layout notes:
- All attention matmuls run on transposed scores s[k, q] so no on-chip
  transposes are needed in the attention inner loop; the only PE transposes
  are q/k tiles ([token, ch] -> [ch, token]) after LayerNorm.
- LayerNorm mean-subtraction is folded into the weights on the host (it is a
  linear map), so on device only an RMS-style rstd = 1/sqrt(mean(q'^2)+eps)
  is needed. rstd is computed as exp(-0.5*ln(var+eps)) because Exp and Ln
  live in the same ACT table set (Sqrt does not), avoiding table thrash.
- The softmax denominator is produced by appending a ones-column to V
  (attn@v then yields numerator rows 0..63 and the denominator in row 64).
- Causality: fully-masked key tiles are skipped by loop bounds, partially
  masked (diagonal) tiles zero the upper triangle of exp(s) via affine_select.
"""

import numpy as np

import concourse.bass as bass
import concourse.mybir as mybir
import concourse.tile as tile
from concourse.bass_utils import run_bass_kernel_spmd
from concourse.masks import make_identity

F32 = mybir.dt.float32
F32R = mybir.dt.float32r

B, S, D, H = 2, 2048, 1024, 16
DH = D // H          # 64
NCORES = 8
HPC = H // NCORES    # 2 heads per core
CH = HPC * DH        # 128 channels per core
T = B * S            # 4096 tokens
DCH = D // 128       # 8 contraction chunks
TT = T // 128        # 32 token tiles
QW = 512             # q-chunk width
QC = S // QW         # 4 q-chunks per batch
KTB = S // 128       # 16 k-tiles per batch
EPS = 1e-5


def _split_drain_waits(nc):
    """walrus in this env only accepts one sync-wait per instruction;
    hoist extra waits onto preceding single-wait NOPs on the same engine."""
    for f in nc.m.functions:
        for blk in f.blocks:
            new_insts = []
            for inst in blk.instructions:
                si = getattr(inst, "sync_info", None)
                if si is not None and si.on_wait and len(si.on_wait) > 1:
                    waits = list(si.on_wait)
                    for j, w in enumerate(waits[:-1]):
                        new_insts.append(
                            mybir.InstNoOp(
                                name=f"{inst.name}-dwsplit{j}",
                                engine=inst.engine,
                                ins=[],
                                outs=[],
                                sync_info=mybir.SyncInfo(on_wait=[w], on_update=[]),
                            )
                        )
                    si.on_wait = [waits[-1]]
                    inst.sync_info = si
                new_insts.append(inst)
            blk.instructions[:] = new_insts


def _build(use_bias=False, pcfg=(4, 0, 2, 2), sbufs=(4, 3, 6, 3, 3, 4)):
    a_bufs, o_bufs, s_bufs, b_bufs = pcfg
    x_bufs, qkv_bufs, ex_bufs, ao_bufs, nrm_bufs, po_bufs = sbufs
    nc = bass.Bass("TRN2", target_bir_lowering=False, debug=False)

    xt_d = nc.dram_tensor("xt", [D, T], F32R, kind="ExternalInput")
    wqkvt_d = nc.dram_tensor("wqkvt", [D, 3 * CH], F32R, kind="ExternalInput")
    bqkv_d = (
        nc.dram_tensor("bqkv", [1, 3 * CH], F32, kind="ExternalInput")
        if use_bias
        else None
    )
    wot_d = nc.dram_tensor("wot", [CH, D], F32R, kind="ExternalInput")
    pot_d = nc.dram_tensor("pot", [D, T], F32, kind="ExternalOutput")

    AF = mybir.ActivationFunctionType
    ALU = mybir.AluOpType

    with tile.TileContext(nc) as tc:
        with (
            tc.tile_pool(name="const", bufs=1) as const_pool,
            tc.tile_pool(name="big", bufs=1) as big,
            tc.tile_pool(name="xt", bufs=x_bufs) as xpool,
            tc.tile_pool(name="qkv", bufs=qkv_bufs) as qkvpool,
            tc.tile_pool(name="ln", bufs=4) as lnpool,
            tc.tile_pool(name="expp", bufs=ex_bufs) as exp_pool,
            tc.tile_pool(name="ao", bufs=ao_bufs) as ao_pool,
            tc.tile_pool(name="nrm", bufs=nrm_bufs) as nrm_pool,
            tc.tile_pool(name="po", bufs=po_bufs) as po_pool,
            tc.tile_pool(name="ps_a", bufs=a_bufs, space="PSUM") as ps_a_pool,
            tc.tile_pool(name="ps_b", bufs=b_bufs, space="PSUM") as ps_b_pool,
            tc.tile_pool(name="ps_s", bufs=s_bufs, space="PSUM") as ps_s_pool,
        ):
            ps_o_pool = (
                tc.alloc_tile_pool(name="ps_o", bufs=o_bufs, space="PSUM")
                if o_bufs
                else None
            )
            identity = const_pool.tile([128, 128], F32)
            make_identity(nc, identity)
            ident_r = const_pool.tile([128, 128], F32R)
            nc.vector.tensor_copy(out=ident_r, in_=identity)

            wqkv_sb = const_pool.tile([128, DCH, 3 * CH], F32R)

            def _load_wqkv(d):
                nc.sync.dma_start(
                    out=wqkv_sb[:, d, :],
                    in_=wqkvt_d[128 * d : 128 * (d + 1), :],
                )

            for d in range(DCH):
                _load_wqkv(d)
            if use_bias:
                bias_sb = const_pool.tile([128, 3 * CH], F32)
                nc.sync.dma_start(
                    out=bias_sb, in_=bqkv_d[0:1, :].to_broadcast([128, 3 * CH])
                )

            qT = big.tile([128, T], F32R)
            kT = big.tile([128, T], F32R)
            vaug = big.tile([128, TT, 2 * (DH + 1)], F32R)
            ones64f = const_pool.tile([1, DH], F32)
            nc.vector.memset(ones64f, 1.0)
            ones64r = const_pool.tile([1, DH], F32R)
            nc.vector.tensor_copy(out=ones64r, in_=ones64f)
            ones32 = const_pool.tile([128, TT, 1], F32)
            nc.vector.memset(ones32, 1.0)
            for oc in (DH, 2 * DH + 1):
                nc.vector.tensor_copy(out=vaug[:, :, oc : oc + 1], in_=ones32)

            # ---- Phase 1: q/k/v projection + LN + transposes ----
            for t in range(TT):
                xt_sb = xpool.tile([128, DCH, 128], F32R, tag="xt")
                hd = DCH // 2
                for part in range(2):
                    nc.sync.dma_start(
                        out=xt_sb[:, part * hd : (part + 1) * hd, :],
                        in_=xt_d[:, 128 * t : 128 * (t + 1)]
                        .rearrange("(a p) t -> p a t", p=128)[
                            :, part * hd : (part + 1) * hd, :
                        ],
                    )
                ps = ps_a_pool.tile([128, 3 * CH], F32, tag="a")
                for d in range(DCH):
                    nc.tensor.matmul(
                        ps,
                        lhsT=xt_sb[:, d, :],
                        rhs=wqkv_sb[:, d, :],
                        start=(d == 0),
                        stop=(d == DCH - 1),
                    )
                if use_bias:
                    qkv = qkvpool.tile([128, 3 * CH], F32, tag="qkv")
                    nc.vector.tensor_add(out=qkv, in0=ps, in1=bias_sb)
                    src_qk = qkv
                else:
                    src_qk = ps

                # RMS-style LN on q and k slices (4 groups of 64)
                sq = lnpool.tile([128, 2 * CH], F32, tag="sq")
                nc.scalar.activation(out=sq, in_=src_qk[:, 0 : 2 * CH], func=AF.Square)
                ssum = lnpool.tile([128, 4], F32, tag="ssum")
                nc.vector.reduce_sum(
                    out=ssum,
                    in_=sq.rearrange("p (g x) -> p g x", x=DH),
                    axis=mybir.AxisListType.X,
                )
                vareps = lnpool.tile([128, 4], F32, tag="vareps")
                nc.vector.tensor_scalar(
                    out=vareps,
                    in0=ssum,
                    scalar1=1.0 / DH,
                    scalar2=EPS,
                    op0=ALU.mult,
                    op1=ALU.add,
                )
                lnv = lnpool.tile([128, 4], F32, tag="lnv")
                nc.scalar.activation(out=lnv, in_=vareps, func=AF.Ln)
                rstd = lnpool.tile([128, 4], F32, tag="rstd")
                nc.scalar.activation(out=rstd, in_=lnv, func=AF.Exp, scale=-0.5)
                qln = qkvpool.tile([128, 2 * CH], F32R, tag="qln")
                rstd_ap = rstd[:, :]
                rstd_b = bass.AP(
                    tensor=rstd_ap.tensor,
                    offset=rstd_ap.offset,
                    ap=rstd_ap.ap + [[0, DH]],
                )
                nc.vector.tensor_mul(
                    out=qln.rearrange("p (g x) -> p g x", x=DH),
                    in0=src_qk[:, 0 : 2 * CH].rearrange("p (g x) -> p g x", x=DH),
                    in1=rstd_b,
                )

                for which, dst in ((0, qT), (1, kT)):
                    pst = ps_b_pool.tile([128, 128], F32R, tag="b")
                    nc.tensor.transpose(
                        pst, qln[:, CH * which : CH * (which + 1)], ident_r
                    )
                    if which == 0:
                        nc.scalar.copy(out=dst[:, 128 * t : 128 * (t + 1)], in_=pst)
                    else:
                        nc.vector.tensor_copy(
                            out=dst[:, 128 * t : 128 * (t + 1)], in_=pst
                        )

                for h in range(HPC):
                    nc.vector.tensor_copy(
                        out=vaug[:, t, (DH + 1) * h : (DH + 1) * h + DH],
                        in_=src_qk[:, 2 * CH + DH * h : 2 * CH + DH * (h + 1)],
                    )

            wo_sb = const_pool.tile([128, D], F32R)
            nc.sync.dma_start(out=wo_sb, in_=wot_d[:, :])

            # ---- Phase 2: per-head causal attention + partial out-projection ----
            for b in range(B):
                for qc in range(QC):
                    q0 = b * S + qc * QW
                    n_kt = (qc + 1) * (QW // 128)
                    ao = ao_pool.tile([128, QW], F32R, tag="ao")
                    for h in range(HPC):
                        if o_bufs:
                            ps_o = ps_o_pool.tile([DH + 1, QW], F32, tag="o")
                        else:
                            ps_o = ps_a_pool.tile([DH + 1, QW], F32, tag="a")
                        for kt in range(n_kt):
                            c0 = max(0, kt * 128 - qc * QW)
                            ps_s = ps_s_pool.tile([128, QW], F32, tag="ps_s")
                            nc.tensor.matmul(
                                ps_s[:, c0:QW],
                                lhsT=kT[
                                    DH * h : DH * (h + 1),
                                    b * S + 128 * kt : b * S + 128 * (kt + 1),
                                ],
                                rhs=qT[DH * h : DH * (h + 1), q0 + c0 : q0 + QW],
                                start=True,
                                stop=True,
                            )
                            ex = exp_pool.tile([128, QW], F32R, tag="ex")
                            nc.scalar.activation(
                                out=ex[:, c0:QW],
                                in_=ps_s[:, c0:QW],
                                func=AF.Exp,
                                scale=1.0 / np.sqrt(DH),
                            )
                            d0 = kt * 128 - qc * QW
                            if d0 >= 0:
                                # diagonal tile: zero exp(s) where k > q
                                nc.gpsimd.affine_select(
                                    out=ex[:, d0 : d0 + 128],
                                    in_=ex[:, d0 : d0 + 128],
                                    compare_op=ALU.is_ge,
                                    fill=0.0,
                                    base=0,
                                    pattern=[[1, 128]],
                                    channel_multiplier=-1,
                                )
                            nc.tensor.matmul(
                                ps_o[:, c0:QW],
                                lhsT=vaug[
                                    :,
                                    b * KTB + kt,
                                    (DH + 1) * h : (DH + 1) * (h + 1),
                                ],
                                rhs=ex[:, c0:QW],
                                start=(kt == 0),
                                stop=(kt == n_kt - 1),
                            )
                        dncp = nrm_pool.tile([1, QW], F32R, tag="dncp")
                        nc.vector.tensor_copy(out=dncp, in_=ps_o[DH : DH + 1, :])
                        psb = ps_b_pool.tile([DH, QW], F32, tag="b")
                        nc.tensor.matmul(
                            psb, lhsT=ones64r, rhs=dncp, start=True, stop=True
                        )
                        rdb = nrm_pool.tile([DH, QW], F32, tag="rdb")
                        nc.vector.reciprocal(out=rdb, in_=psb)
                        nc.vector.tensor_mul(
                            out=ao[DH * h : DH * (h + 1), :],
                            in0=ps_o[0:DH, :],
                            in1=rdb,
                        )
                    for dc in range(DCH):
                        ps_po = ps_b_pool.tile([128, QW], F32, tag="b")
                        nc.tensor.matmul(
                            ps_po,
                            lhsT=wo_sb[:, 128 * dc : 128 * (dc + 1)],
                            rhs=ao,
                            start=True,
                            stop=True,
                        )
                        po_sb = po_pool.tile([128, QW], F32, tag="po")
                        nc.vector.tensor_copy(out=po_sb, in_=ps_po)
                        nc.sync.dma_start(
                            out=pot_d[128 * dc : 128 * (dc + 1), q0 : q0 + QW],
                            in_=po_sb,
                        )
            if ps_o_pool is not None:
                ps_o_pool.release()

    _split_drain_waits(nc)
    return nc


_NC_CACHE = {}


def _get_nc(use_bias=False):
    if use_bias not in _NC_CACHE:
        _NC_CACHE[use_bias] = _build(use_bias)
    return _NC_CACHE[use_bias]


def _prep_inputs(x, Wq, bq, Wk, bk, Wv, bv, Wo):
    xt = np.ascontiguousarray(x.reshape(T, D).T).astype(np.float32)
    in_maps = []
    for c in range(NCORES):
        sl = slice(CH * c, CH * (c + 1))
        wq_c = np.array(Wq[sl, :], dtype=np.float32)
        bq_c = np.array(bq[sl], dtype=np.float32)
        wk_c = np.array(Wk[sl, :], dtype=np.float32)
        bk_c = np.array(bk[sl], dtype=np.float32)
        # fold the LayerNorm mean-subtraction (a linear map) into W and b
        for h in range(HPC):
            blk = slice(DH * h, DH * (h + 1))
            wq_c[blk, :] -= wq_c[blk, :].mean(axis=0, keepdims=True)
            bq_c[blk] -= bq_c[blk].mean()
            wk_c[blk, :] -= wk_c[blk, :].mean(axis=0, keepdims=True)
            bk_c[blk] -= bk_c[blk].mean()
        wv_c = np.array(Wv[sl, :], dtype=np.float32)
        bv_c = np.array(bv[sl], dtype=np.float32)
        wqkvt = np.ascontiguousarray(
            np.concatenate([wq_c, wk_c, wv_c], axis=0).T
        ).astype(np.float32)
        bqkv = np.concatenate([bq_c, bk_c, bv_c])[None, :].astype(np.float32)
        wot = np.ascontiguousarray(Wo[:, sl].T).astype(np.float32)
        in_maps.append({"xt": xt, "wqkvt": wqkvt, "bqkv": bqkv, "wot": wot})
    return in_maps


def kernel(x, mask, Wq, bq, Wk, bk, Wv, bv, Wo, bo, _trace=False):
    x = np.asarray(x, dtype=np.float32)
    in_maps = _prep_inputs(
        x,
        np.asarray(Wq),
        np.asarray(bq),
        np.asarray(Wk),
        np.asarray(bk),
        np.asarray(Wv),
        np.asarray(bv),
        np.asarray(Wo),
    )
    use_bias = bool(
        np.any(np.asarray(bq)) or np.any(np.asarray(bk)) or np.any(np.asarray(bv))
    )
    nc = _get_nc(use_bias)
    res = run_bass_kernel_spmd(
        nc, in_maps, core_ids=list(range(NCORES)), trace=_trace
    )
    pot = np.zeros((D, T), np.float64)
    for c in range(NCORES):
        pot += res.results[c]["pot"].astype(np.float64)
    out = pot.T.astype(np.float32) + np.asarray(bo, dtype=np.float32)[None, :]
    out = out.reshape(B, S, D)
    if _trace:
        return out, res
    return out

